# revision 8
# baseline (speedup 1.0000x reference)
"""FFM layer (linear + field-aware FM interaction) on 8 Trainium2 cores.

Sharding: row-parallel GEMM over the feature axis. Core c holds a
13056-feature stripe of inputs^T ([13056, 1024]) and of the combined
weight matrix G = [v.reshape(F, 312) | w] ([13056, 313]). Each core
computes its partial inputs_c^T.T @ G_c -> [1024, 313] with fp32
(float32r PE mode) matmuls accumulated in PSUM over 102 k-tiles.
The host sums the 8 partials and applies the cheap FM epilogue
(sum-square identity) in fp64, returning [1024, 1] fp32.
"""

import numpy as np

B = 1024
F = 104013
FIELD = 39
K = 8
NV = FIELD * K          # 312 interaction columns
NL = NV                 # linear column index
NK = NV + 2             # + linear column + 1 zero pad col (fp32r needs even N)
N_CORES = 8
KT = 102                # 128-row k-tiles per core
FPC = KT * 128          # 13056 padded features per core
CH = 3                  # k-tiles per DMA chunk
BUFS = 6                # SBUF double-buffer depth for streamed chunks
DMA_ENGINE = "sync"     # "sync" (HWDGE) or "gpsimd" (SWDGE)

_nc = None
last_exec_time_ns = None


def _build():
    from concourse import bass, mybir, tile, bacc

    nc = bacc.Bacc("TRN2", num_devices=N_CORES)
    f32 = mybir.dt.float32
    f32r = mybir.dt.float32r

    xt = nc.dram_tensor("xt", [FPC, B], f32r, kind="ExternalInput")
    g = nc.dram_tensor("g", [FPC, NK], f32r, kind="ExternalInput")
    out = nc.dram_tensor("out", [B, NK], f32, kind="ExternalOutput")

    xt_r = xt.rearrange("(t p) m -> p t m", p=128)  # [128, KT, B]
    g_r = g.rearrange("(t p) n -> p t n", p=128)    # [128, KT, NK]

    with tile.TileContext(nc) as tc:
        with (
            tc.tile_pool(name="xt", bufs=BUFS) as xt_pool,
            tc.tile_pool(name="g", bufs=BUFS) as g_pool,
            tc.tile_pool(name="acc", bufs=1, space=bass.MemorySpace.PSUM) as psum_pool,
            tc.tile_pool(name="o", bufs=1) as out_pool,
        ):
            n_b = B // 128
            accs = [
                psum_pool.tile([128, NK], f32, tag=f"acc{b}", name=f"acc{b}")
                for b in range(n_b)
            ]
            dma = nc.sync if DMA_ENGINE == "sync" else nc.gpsimd
            # Graduated chunks: tiny first chunks so the PE starts as soon
            # as possible, steady CH-tile chunks afterwards.
            chunks = []
            for n in [1, 1, 2, 2]:
                if sum(chunks) + n <= KT:
                    chunks.append(n)
            while KT - sum(chunks) > 0:
                chunks.append(min(CH, KT - sum(chunks)))
            kc = 0
            for ci, n in enumerate(chunks):
                last_chunk = ci == len(chunks) - 1
                xt_t = xt_pool.tile([128, n, B], f32r, tag="xt", name=f"xt{kc}")
                dma.dma_start(xt_t[:], xt_r[:, kc : kc + n, :])
                g_t = g_pool.tile([128, n, NK], f32r, tag="g", name=f"gt{kc}")
                dma.dma_start(g_t[:], g_r[:, kc : kc + n, :])
                # b-major in the last chunk so each acc finishes (and its
                # copy-out can start) as early as possible.
                order = (
                    [(i, b) for b in range(n_b) for i in range(n)]
                    if last_chunk
                    else [(i, b) for i in range(n) for b in range(n_b)]
                )
                for i, b in order:
                    k = kc + i
                    nc.tensor.matmul(
                        accs[b][:],
                        xt_t[:, i, b * 128 : (b + 1) * 128],
                        g_t[:, i, :],
                        start=(k == 0),
                        stop=(k == KT - 1),
                    )
                kc += n
            for b in range(n_b):
                o = out_pool.tile([128, NK], f32, tag=f"o{b}", name=f"ot{b}")
                nc.vector.tensor_copy(o[:], accs[b][:])
                dma.dma_start(out[b * 128 : (b + 1) * 128, :], o[:])
    nc.compile()
    return nc


def _get_nc():
    global _nc
    if _nc is None:
        _nc = _build()
    return _nc


def kernel(inputs, w0, w, v, _trace=False):
    global last_exec_time_ns
    from concourse.bass_utils import run_bass_kernel_spmd

    inputs = np.asarray(inputs, dtype=np.float32)
    w0 = np.asarray(w0, dtype=np.float32)
    w = np.asarray(w, dtype=np.float32)
    v = np.asarray(v, dtype=np.float32)

    # G = [v | w] : [F, 313], zero-padded to 8 * 13056 rows
    G = np.zeros((N_CORES * FPC, NK), dtype=np.float32)
    G[:F, :NV] = v.reshape(F, NV)
    G[:F, NL] = w[:, 0]
    # inputs^T, zero-padded the same way
    XT = np.zeros((N_CORES * FPC, B), dtype=np.float32)
    XT[:F] = inputs.T

    in_maps = [
        {"xt": XT[c * FPC : (c + 1) * FPC], "g": G[c * FPC : (c + 1) * FPC]}
        for c in range(N_CORES)
    ]
    nc = _get_nc()
    res = run_bass_kernel_spmd(nc, in_maps, list(range(N_CORES)), trace=_trace)
    last_exec_time_ns = res.exec_time_ns

    total = np.zeros((B, NK), dtype=np.float64)
    for c in range(N_CORES):
        total += res.results[c]["out"]

    field_f = total[:, :NV].reshape(B, FIELD, K)
    linear = total[:, NL] + np.float64(w0[0])
    s = field_f.sum(axis=1)                                     # [B, K]
    inter = 0.5 * ((s * s).sum(axis=-1) - (field_f * field_f).sum(axis=(1, 2)))
    return (linear + inter)[:, None].astype(np.float32)


# revision 10
# speedup vs baseline: 1.0890x; 1.0890x over previous
"""FFM layer (linear + field-aware FM interaction) on 8 Trainium2 cores.

Sharding: row-parallel GEMM over the feature axis. Core c holds a
13056-feature stripe of inputs^T ([13056, 1024]) and of the combined
weight matrix G = [v.reshape(F, 312) | w] ([13056, 313]). Each core
computes its partial inputs_c^T.T @ G_c -> [1024, 313] with fp32
(float32r PE mode) matmuls accumulated in PSUM over 102 k-tiles.
The host sums the 8 partials and applies the cheap FM epilogue
(sum-square identity) in fp64, returning [1024, 1] fp32.
"""

import numpy as np

B = 1024
F = 104013
FIELD = 39
K = 8
NV = FIELD * K          # 312 interaction columns
NL = NV                 # linear column index
NK = NV + 2             # + linear column + 1 zero pad col (fp32r needs even N)
N_CORES = 8
KT = 102                # 128-row k-tiles per core
FPC = KT * 128          # 13056 padded features per core
CH = 3                  # k-tiles per DMA chunk
BUFS = 6                # SBUF double-buffer depth for streamed chunks
DMA_ENGINE = "sync"     # "sync" (HWDGE) or "gpsimd" (SWDGE)

_nc = None
last_exec_time_ns = None


def _build():
    from concourse import bass, mybir, tile, bacc

    nc = bacc.Bacc("TRN2", num_devices=N_CORES)
    f32 = mybir.dt.float32
    f32r = mybir.dt.float32r

    xt = nc.dram_tensor("xt", [FPC, B], f32r, kind="ExternalInput")
    g = nc.dram_tensor("g", [FPC, NK], f32r, kind="ExternalInput")
    out = nc.dram_tensor("out", [B, NK], f32, kind="ExternalOutput")

    xt_r = xt.rearrange("(t p) m -> p t m", p=128)  # [128, KT, B]
    g_r = g.rearrange("(t p) n -> p t n", p=128)    # [128, KT, NK]

    with tile.TileContext(nc) as tc:
        with (
            tc.tile_pool(name="xt", bufs=BUFS) as xt_pool,
            tc.tile_pool(name="g", bufs=BUFS) as g_pool,
            tc.tile_pool(name="acc", bufs=1, space=bass.MemorySpace.PSUM) as psum_pool,
            tc.tile_pool(name="o", bufs=1) as out_pool,
        ):
            n_b = B // 128
            accs = [
                psum_pool.tile([128, NK], f32, tag=f"acc{b}", name=f"acc{b}")
                for b in range(n_b)
            ]
            dma = nc.sync if DMA_ENGINE == "sync" else nc.gpsimd
            # Graduated chunks: tiny first chunks so the PE starts as soon
            # as possible, steady CH-tile chunks afterwards.
            chunks = []
            for n in [1, 1, 2, 2]:
                if sum(chunks) + n <= KT:
                    chunks.append(n)
            while KT - sum(chunks) > 0:
                chunks.append(min(CH, KT - sum(chunks)))
            kc = 0
            for ci, n in enumerate(chunks):
                last_chunk = ci == len(chunks) - 1
                g_t = g_pool.tile([128, n, NK], f32r, tag="g", name=f"gt{kc}")
                dma.dma_start(g_t[:], g_r[:, kc : kc + n, :])
                xt_ts = []
                for i in range(n):
                    xt_i = xt_pool.tile([128, B], f32r, tag=f"xt{i}", name=f"xt{kc}_{i}")
                    dma.dma_start(xt_i[:], xt_r[:, kc + i, :])
                    xt_ts.append(xt_i)
                # b-major in the last chunk so each acc finishes (and its
                # copy-out can start) as early as possible.
                order = (
                    [(i, b) for b in range(n_b) for i in range(n)]
                    if last_chunk
                    else [(i, b) for i in range(n) for b in range(n_b)]
                )
                for i, b in order:
                    k = kc + i
                    nc.tensor.matmul(
                        accs[b][:],
                        xt_ts[i][:, b * 128 : (b + 1) * 128],
                        g_t[:, i, :],
                        start=(k == 0),
                        stop=(k == KT - 1),
                    )
                kc += n
            for b in range(n_b):
                o = out_pool.tile([128, NK], f32, tag=f"o{b}", name=f"ot{b}")
                nc.vector.tensor_copy(o[:], accs[b][:])
                dma.dma_start(out[b * 128 : (b + 1) * 128, :], o[:])
    nc.compile()
    return nc


def _get_nc():
    global _nc
    if _nc is None:
        _nc = _build()
    return _nc


def kernel(inputs, w0, w, v, _trace=False):
    global last_exec_time_ns
    from concourse.bass_utils import run_bass_kernel_spmd

    inputs = np.asarray(inputs, dtype=np.float32)
    w0 = np.asarray(w0, dtype=np.float32)
    w = np.asarray(w, dtype=np.float32)
    v = np.asarray(v, dtype=np.float32)

    # G = [v | w] : [F, 313], zero-padded to 8 * 13056 rows
    G = np.zeros((N_CORES * FPC, NK), dtype=np.float32)
    G[:F, :NV] = v.reshape(F, NV)
    G[:F, NL] = w[:, 0]
    # inputs^T, zero-padded the same way
    XT = np.zeros((N_CORES * FPC, B), dtype=np.float32)
    XT[:F] = inputs.T

    in_maps = [
        {"xt": XT[c * FPC : (c + 1) * FPC], "g": G[c * FPC : (c + 1) * FPC]}
        for c in range(N_CORES)
    ]
    nc = _get_nc()
    res = run_bass_kernel_spmd(nc, in_maps, list(range(N_CORES)), trace=_trace)
    last_exec_time_ns = res.exec_time_ns

    total = np.zeros((B, NK), dtype=np.float64)
    for c in range(N_CORES):
        total += res.results[c]["out"]

    field_f = total[:, :NV].reshape(B, FIELD, K)
    linear = total[:, NL] + np.float64(w0[0])
    s = field_f.sum(axis=1)                                     # [B, K]
    inter = 0.5 * ((s * s).sum(axis=-1) - (field_f * field_f).sum(axis=(1, 2)))
    return (linear + inter)[:, None].astype(np.float32)


# revision 13
# speedup vs baseline: 1.1897x; 1.0925x over previous
"""FFM layer (linear + field-aware FM interaction) on 8 Trainium2 cores.

Sharding: row-parallel GEMM over the feature axis. Core c holds a
13056-feature stripe of inputs^T ([13056, 1024]) and of the combined
weight matrix G = [v.reshape(F, 312) | w] ([13056, 313]). Each core
computes its partial inputs_c^T.T @ G_c -> [1024, 313] with fp32
(float32r PE mode) matmuls accumulated in PSUM over 102 k-tiles.
The host sums the 8 partials and applies the cheap FM epilogue
(sum-square identity) in fp64, returning [1024, 1] fp32.
"""

import numpy as np

B = 1024
F = 104013
FIELD = 39
K = 8
NV = FIELD * K          # 312 interaction columns
NL = NV                 # linear column index
NK = NV + 2             # + linear column + 1 zero pad col (fp32r needs even N)
N_CORES = 8
KT = 102                # 128-row k-tiles per core
FPC = KT * 128          # 13056 padded features per core
CH = 3                  # k-tiles per DMA chunk
BUFS = 6                # SBUF double-buffer depth for streamed chunks
DMA_ENGINE = "sync"     # "sync" (HWDGE) or "gpsimd" (SWDGE)
WARMUP_LDW = 40         # dummy ldweights before the stream (PE pre-warm)
FILLER_LDW = 8          # dummy ldweights per chunk (keep HAM warm in stalls)

_nc = None
last_exec_time_ns = None


def _build():
    from concourse import bass, mybir, tile, bacc

    nc = bacc.Bacc("TRN2", num_devices=N_CORES)
    f32 = mybir.dt.float32
    f32r = mybir.dt.float32r

    xt = nc.dram_tensor("xt", [FPC, B], f32r, kind="ExternalInput")
    g = nc.dram_tensor("g", [FPC, NK], f32r, kind="ExternalInput")
    out = nc.dram_tensor("out", [B, NK], f32, kind="ExternalOutput")

    xt_r = xt.rearrange("(t p) m -> p t m", p=128)  # [128, KT, B]
    g_r = g.rearrange("(t p) n -> p t n", p=128)    # [128, KT, NK]

    with tile.TileContext(nc) as tc:
        with (
            tc.tile_pool(name="xt", bufs=BUFS) as xt_pool,
            tc.tile_pool(name="g", bufs=BUFS) as g_pool,
            tc.tile_pool(name="acc", bufs=1, space=bass.MemorySpace.PSUM) as psum_pool,
            tc.tile_pool(name="o", bufs=1) as out_pool,
        ):
            n_b = B // 128
            accs = [
                psum_pool.tile([128, NK], f32, tag=f"acc{b}", name=f"acc{b}")
                for b in range(n_b)
            ]
            # Scratch bf16 weight tile: dummy ldweights on it keep the PE
            # HAM activity monitor warm during DMA stalls. The loaded
            # weights are never used (every real fp32r matmul self-loads).
            bf16 = mybir.dt.bfloat16
            warm = out_pool.tile([128, 128], bf16, tag="warm", name="warm")
            nc.gpsimd.memset(warm[:], 0.0)
            for _ in range(WARMUP_LDW):
                nc.tensor.ldweights(warm[:])
            dma = nc.sync if DMA_ENGINE == "sync" else nc.gpsimd
            # Graduated chunks: tiny first chunks so the PE starts as soon
            # as possible, steady CH-tile chunks afterwards.
            chunks = []
            for n in [1, 1, 2, 2]:
                if sum(chunks) + n <= KT:
                    chunks.append(n)
            while KT - sum(chunks) > 0:
                chunks.append(min(CH, KT - sum(chunks)))
            kc = 0
            for ci, n in enumerate(chunks):
                last_chunk = ci == len(chunks) - 1
                xt_t = xt_pool.tile([128, n, B], f32r, tag="xt", name=f"xt{kc}")
                dma.dma_start(xt_t[:], xt_r[:, kc : kc + n, :])
                g_t = g_pool.tile([128, n, NK], f32r, tag="g", name=f"gt{kc}")
                dma.dma_start(g_t[:], g_r[:, kc : kc + n, :])
                # b-major in the last chunk so each acc finishes (and its
                # copy-out can start) as early as possible.
                order = (
                    [(i, b) for b in range(n_b) for i in range(n)]
                    if last_chunk
                    else [(i, b) for i in range(n) for b in range(n_b)]
                )
                for i, b in order:
                    k = kc + i
                    nc.tensor.matmul(
                        accs[b][:],
                        xt_t[:, i, b * 128 : (b + 1) * 128],
                        g_t[:, i, :],
                        start=(k == 0),
                        stop=(k == KT - 1),
                    )
                if not last_chunk:
                    for _ in range(FILLER_LDW):
                        nc.tensor.ldweights(warm[:])
                kc += n
            for b in range(n_b):
                o = out_pool.tile([128, NK], f32, tag=f"o{b}", name=f"ot{b}")
                nc.vector.tensor_copy(o[:], accs[b][:])
                dma.dma_start(out[b * 128 : (b + 1) * 128, :], o[:])
    nc.compile()
    return nc


def _get_nc():
    global _nc
    if _nc is None:
        _nc = _build()
    return _nc


def kernel(inputs, w0, w, v, _trace=False):
    global last_exec_time_ns
    from concourse.bass_utils import run_bass_kernel_spmd

    inputs = np.asarray(inputs, dtype=np.float32)
    w0 = np.asarray(w0, dtype=np.float32)
    w = np.asarray(w, dtype=np.float32)
    v = np.asarray(v, dtype=np.float32)

    # G = [v | w] : [F, 313], zero-padded to 8 * 13056 rows
    G = np.zeros((N_CORES * FPC, NK), dtype=np.float32)
    G[:F, :NV] = v.reshape(F, NV)
    G[:F, NL] = w[:, 0]
    # inputs^T, zero-padded the same way
    XT = np.zeros((N_CORES * FPC, B), dtype=np.float32)
    XT[:F] = inputs.T

    in_maps = [
        {"xt": XT[c * FPC : (c + 1) * FPC], "g": G[c * FPC : (c + 1) * FPC]}
        for c in range(N_CORES)
    ]
    nc = _get_nc()
    res = run_bass_kernel_spmd(nc, in_maps, list(range(N_CORES)), trace=_trace)
    last_exec_time_ns = res.exec_time_ns

    total = np.zeros((B, NK), dtype=np.float64)
    for c in range(N_CORES):
        total += res.results[c]["out"]

    field_f = total[:, :NV].reshape(B, FIELD, K)
    linear = total[:, NL] + np.float64(w0[0])
    s = field_f.sum(axis=1)                                     # [B, K]
    inter = 0.5 * ((s * s).sum(axis=-1) - (field_f * field_f).sum(axis=(1, 2)))
    return (linear + inter)[:, None].astype(np.float32)


# revision 16
# speedup vs baseline: 1.1982x; 1.0072x over previous
"""FFM layer (linear + field-aware FM interaction) on 8 Trainium2 cores.

Sharding: row-parallel GEMM over the feature axis. Core c holds a
13056-feature stripe of inputs^T ([13056, 1024]) and of the combined
weight matrix G = [v.reshape(F, 312) | w] ([13056, 313]). Each core
computes its partial inputs_c^T.T @ G_c -> [1024, 313] with fp32
(float32r PE mode) matmuls accumulated in PSUM over 102 k-tiles.
The host sums the 8 partials and applies the cheap FM epilogue
(sum-square identity) in fp64, returning [1024, 1] fp32.
"""

import numpy as np

B = 1024
F = 104013
FIELD = 39
K = 8
NV = FIELD * K          # 312 interaction columns
NL = NV                 # linear column index
NK = NV + 2             # + linear column + 1 zero pad col (fp32r needs even N)
N_CORES = 8
KT = 102                # 128-row k-tiles per core
FPC = KT * 128          # 13056 padded features per core
CH = 3                  # k-tiles per DMA chunk
BUFS = 6                # SBUF double-buffer depth for streamed chunks
DMA_ENGINE = "sync"     # "sync" (HWDGE) or "gpsimd" (SWDGE)
WARMUP_LDW = 0          # dummy ldweights before the stream (PE pre-warm)
FILLER_LDW = 0          # dummy ldweights per chunk (keep HAM warm in stalls)
G_DMA = "sync"          # engine for g-stream DMAs
OUT_DMA = "sync"        # engine for output DMAs

_nc = None
last_exec_time_ns = None


def _build():
    from concourse import bass, mybir, tile, bacc

    nc = bacc.Bacc("TRN2", num_devices=N_CORES)
    f32 = mybir.dt.float32
    f32r = mybir.dt.float32r

    xt = nc.dram_tensor("xt", [FPC, B], f32r, kind="ExternalInput")
    g = nc.dram_tensor("g", [FPC, NK], f32r, kind="ExternalInput")
    out = nc.dram_tensor("out", [B, NK], f32, kind="ExternalOutput")

    xt_r = xt.rearrange("(t p) m -> p t m", p=128)  # [128, KT, B]
    g_r = g.rearrange("(t p) n -> p t n", p=128)    # [128, KT, NK]

    with tile.TileContext(nc) as tc:
        with (
            tc.tile_pool(name="xt", bufs=BUFS) as xt_pool,
            tc.tile_pool(name="g", bufs=BUFS) as g_pool,
            tc.tile_pool(name="acc", bufs=1, space=bass.MemorySpace.PSUM) as psum_pool,
            tc.tile_pool(name="o", bufs=1) as out_pool,
        ):
            n_b = B // 128
            accs = [
                psum_pool.tile([128, NK], f32, tag=f"acc{b}", name=f"acc{b}")
                for b in range(n_b)
            ]
            # Scratch bf16 weight tile: dummy ldweights on it keep the PE
            # HAM activity monitor warm during DMA stalls. The loaded
            # weights are never used (every real fp32r matmul self-loads).
            if WARMUP_LDW or FILLER_LDW:
                bf16 = mybir.dt.bfloat16
                warm = out_pool.tile([128, 128], bf16, tag="warm", name="warm")
                nc.gpsimd.memset(warm[:], 0.0)
                for _ in range(WARMUP_LDW):
                    nc.tensor.ldweights(warm[:])
            dma = nc.sync if DMA_ENGINE == "sync" else nc.gpsimd
            dma_g = nc.sync if G_DMA == "sync" else nc.gpsimd
            dma_out = nc.sync if OUT_DMA == "sync" else nc.gpsimd
            # Graduated chunks: tiny first chunks so the PE starts as soon
            # as possible, steady CH-tile chunks afterwards.
            chunks = []
            for n in [1, 1, 2, 2]:
                if sum(chunks) + n <= KT:
                    chunks.append(n)
            while KT - sum(chunks) > 0:
                chunks.append(min(CH, KT - sum(chunks)))
            kc = 0
            for ci, n in enumerate(chunks):
                last_chunk = ci == len(chunks) - 1
                xt_t = xt_pool.tile([128, n, B], f32r, tag="xt", name=f"xt{kc}")
                dma.dma_start(xt_t[:], xt_r[:, kc : kc + n, :])
                g_t = g_pool.tile([128, n, NK], f32r, tag="g", name=f"gt{kc}")
                dma_g.dma_start(g_t[:], g_r[:, kc : kc + n, :])
                # b-major in the last chunk so each acc finishes (and its
                # copy-out can start) as early as possible.
                order = (
                    [(i, b) for b in range(n_b) for i in range(n)]
                    if last_chunk
                    else [(i, b) for i in range(n) for b in range(n_b)]
                )
                for i, b in order:
                    k = kc + i
                    nc.tensor.matmul(
                        accs[b][:],
                        xt_t[:, i, b * 128 : (b + 1) * 128],
                        g_t[:, i, :],
                        start=(k == 0),
                        stop=(k == KT - 1),
                    )
                if FILLER_LDW and not last_chunk:
                    for _ in range(FILLER_LDW):
                        nc.tensor.ldweights(warm[:])
                kc += n
            for b in range(n_b):
                o = out_pool.tile([128, NK], f32, tag=f"o{b}", name=f"ot{b}")
                nc.vector.tensor_copy(o[:], accs[b][:])
                dma_out.dma_start(out[b * 128 : (b + 1) * 128, :], o[:])
    nc.compile()
    return nc


def _get_nc():
    global _nc
    if _nc is None:
        _nc = _build()
    return _nc


def kernel(inputs, w0, w, v, _trace=False):
    global last_exec_time_ns
    from concourse.bass_utils import run_bass_kernel_spmd

    inputs = np.asarray(inputs, dtype=np.float32)
    w0 = np.asarray(w0, dtype=np.float32)
    w = np.asarray(w, dtype=np.float32)
    v = np.asarray(v, dtype=np.float32)

    # G = [v | w] : [F, 313], zero-padded to 8 * 13056 rows
    G = np.zeros((N_CORES * FPC, NK), dtype=np.float32)
    G[:F, :NV] = v.reshape(F, NV)
    G[:F, NL] = w[:, 0]
    # inputs^T, zero-padded the same way
    XT = np.zeros((N_CORES * FPC, B), dtype=np.float32)
    XT[:F] = inputs.T

    in_maps = [
        {"xt": XT[c * FPC : (c + 1) * FPC], "g": G[c * FPC : (c + 1) * FPC]}
        for c in range(N_CORES)
    ]
    nc = _get_nc()
    import os

    prev = os.environ.get("BASS_NEVER_TRACE")
    if not _trace:
        # Profiling needs an NTFF hook this container may not have; make
        # sure a stray BASS_TRACE env var can't pull us down that path.
        os.environ["BASS_NEVER_TRACE"] = "1"
    try:
        res = run_bass_kernel_spmd(nc, in_maps, list(range(N_CORES)), trace=_trace)
    finally:
        if not _trace:
            if prev is None:
                os.environ.pop("BASS_NEVER_TRACE", None)
            else:
                os.environ["BASS_NEVER_TRACE"] = prev
    last_exec_time_ns = res.exec_time_ns

    total = np.zeros((B, NK), dtype=np.float64)
    for c in range(N_CORES):
        total += res.results[c]["out"]

    field_f = total[:, :NV].reshape(B, FIELD, K)
    linear = total[:, NL] + np.float64(w0[0])
    s = field_f.sum(axis=1)                                     # [B, K]
    inter = 0.5 * ((s * s).sum(axis=-1) - (field_f * field_f).sum(axis=(1, 2)))
    return (linear + inter)[:, None].astype(np.float32)
